# revision 29
# baseline (speedup 1.0000x reference)
"""MinusAttention kernel for Trainium2 (8 NeuronCores, Bass/Tile).

Math: score[i,j] = (w.q_i - w.k_j + b) / sqrt(E) with causal mask.
Within a softmax row i the w.q_i and b terms are constant across j and
cancel, so

    weights[i,j] = g_j / sum_{j'<=i} g_j',   g_j = exp(-w.k_j / sqrt(E))
    out[i,:]     = (sum_{j<=i} g_j V[j,:]) / (sum_{j<=i} g_j)

i.e. a causal cumulative weighted average of V -- O(S*E) per (b,h) --
and the output does not depend on queries at all.

Device kernel per core (4 of the 32 (b,h) pairs), all fp16 IO:

  s = 128*k + (127 - row): row-REVERSED within each 128-block, so each
  block's running total lands on PSUM row 0.  Prefix = lower-triangular
  fp16 matmul (within block) + per-block carry matmuls.

  Per pair p (its own small DMA so compute starts ~6us in):
    sk = sum_e kt (two fp16 halving adds + fp32 reduce, DVE)
    g = exp(sk) (ACT) -> wg = v*g (DVE) ->
    2x 512-col prefix matmuls into 2 PSUM banks (PE, fp16 1c/row)
    block totals = PSUM row 0: ACT copies [1,(k,e)] -> SBUF fp16,
    SBUF->SBUF DMA scatters them onto 16 partitions (bs)
    rm = strict-lower-mask * bs (DVE 2x, fp16 packed)
    carry matmuls ones16 @ rm accumulate into the same banks
    pairs 0-1: ACT drains PSUM->fp16 cw, out = cw * (1/den) (DVE)
    pairs 2-3: out = PSUM * (1/den) fused on DVE (one hop less; these
    pairs are tail-critical)

  Denominator: per-pair prefix matmuls of g into a per-duo PSUM bank
  (start=True on the even pair resets the bank -- hardware resets the
  WHOLE bank, so duos cannot share one), row-0 extract + scatter, drm
  carry, reciprocal -> r[128, pair, k] fp16.

  Scheduling: all constants are host-baked (one small DMA); 6 junk
  matmuls into the den banks warm the PE p-state during the input
  DMAs; order-only deps keep the DVE stream chain-major; pair 3 reuses
  pair 0's PSUM banks (pool rotation) and its matmul/extract/scatter
  are emitted before the output DMAs to avoid queue head-of-line
  blocking; rm ops for all pairs precede all finals in the DVE stream.

  Measured on trn2: 55.6us (baseline fp32 kernel) -> ~37us; output
  rel err ~6e-4 (fp16 quantization).
"""
import numpy as np

B, L, S, H, E = 4, 2048, 2048, 8, 64
NCORES = 8
PAIRS = (B * H) // NCORES  # 4 (b,h) pairs per core
NBLK = S // 128  # 16 blocks of 128 positions
DUOS = PAIRS // 2  # pairs processed two at a time
SCALE = np.float32(1.0 / np.sqrt(np.float32(E)))

# consts column map: triL is its own [128,128] tensor; the 16-partition
# masks live in a second [16, CWID2] tensor
CW_ONES = 0  # ones16 [16, 0:128]
CW_MASK3 = 128  # mask3 [16, 128:1152]  (k', k, e) strict-lower
CW_MASK3D = 1152  # mask3d [16, 1152:1184]  (k', j, k) strict-lower
CWID2 = 1184

TRACE = False
LAST_RESULTS = None

_compiled = None


def _consts_host():
    tri = np.tril(np.ones((128, 128), np.float16))  # p' >= p (row-reversed)
    c = np.zeros((16, CWID2), dtype=np.float16)
    c[:, CW_ONES : CW_ONES + 128] = 1.0
    kp = np.arange(16)
    m3 = (kp[:, None] < kp[None, :]).astype(np.float16)  # [k', k]
    c[:, CW_MASK3 : CW_MASK3 + NBLK * E] = np.repeat(m3, E, axis=1)
    c[:, CW_MASK3D : CW_MASK3D + 2 * NBLK] = np.concatenate([m3, m3], axis=1)
    return tri, c


def _build():
    from concourse import bacc
    import concourse.mybir as mybir
    import concourse.tile as tile
    from concourse.tile_rust import add_dep_helper

    f16 = mybir.dt.float16
    f32 = mybir.dt.float32
    nc = bacc.Bacc("TRN2", target_bir_lowering=False, debug=False)

    # per-pair tensors, (row, ...) with 2KB contiguous partition lines
    ktin = nc.dram_tensor("ktin", [PAIRS, 128, NBLK, E], f16, kind="ExternalInput")
    vin = nc.dram_tensor("vin", [PAIRS, 128, NBLK, E], f16, kind="ExternalInput")
    cinA = nc.dram_tensor("cinA", [128, 128], f16, kind="ExternalInput")
    cinB = nc.dram_tensor("cinB", [16, CWID2], f16, kind="ExternalInput")
    outT = nc.dram_tensor("outT", [PAIRS, 128, NBLK, E], f16, kind="ExternalOutput")

    with tile.TileContext(nc) as tc:
        with (
            tc.tile_pool(name="const", bufs=1) as cpool,
            tc.tile_pool(name="ktp", bufs=PAIRS) as ktp,
            tc.tile_pool(name="vp", bufs=PAIRS) as vp,
            tc.tile_pool(name="s1p", bufs=2) as s1p,
            tc.tile_pool(name="s2p", bufs=2) as s2p,
            tc.tile_pool(name="skp", bufs=2) as skp,
            tc.tile_pool(name="wgp", bufs=PAIRS) as wgp,
            tc.tile_pool(name="bs1p", bufs=PAIRS) as bs1p,
            tc.tile_pool(name="bsp", bufs=PAIRS) as bsp,
            tc.tile_pool(name="rmp", bufs=PAIRS) as rmp,
            tc.tile_pool(name="dbs1p", bufs=2) as dbs1p,
            tc.tile_pool(name="dbsp", bufs=2) as dbsp,
            tc.tile_pool(name="drmp", bufs=2) as drmp,
            tc.tile_pool(name="cwp", bufs=2) as cwp,
            tc.tile_pool(name="otp", bufs=PAIRS) as otp,
            tc.tile_pool(name="psp", bufs=3, space="PSUM") as psp,
            tc.tile_pool(name="dpsp", bufs=2, space="PSUM") as dpsp,
        ):
            ctri = cpool.tile([128, 128], f16)
            cmsk = cpool.tile([16, CWID2], f16)
            triL = ctri[:]
            ones16 = cmsk[:, CW_ONES : CW_ONES + 128]
            mask3 = cmsk[:, CW_MASK3 : CW_MASK3 + NBLK * E].rearrange(
                "p (k e) -> p k e", k=NBLK
            )
            mask3d = cmsk[:, CW_MASK3D : CW_MASK3D + 2 * NBLK].rearrange(
                "p (j k) -> p j k", j=2
            )

            G = cpool.tile([128, PAIRS, NBLK], f16)
            r = cpool.tile([128, PAIRS, NBLK], f16)
            # full-bank tiles: also the target of PE warm-up matmuls; the
            # real den prefix matmul (start=True) resets the bank
            denbank = [
                dpsp.tile([128, 512], f32, tag="den", name=f"denb{d}")
                for d in range(DUOS)
            ]
            dens = [
                denbank[d][:, 0 : 2 * NBLK].rearrange("p (j k) -> p j k", j=2)
                for d in range(DUOS)
            ]

            # --- inputs: per-pair DMAs so compute starts after ~0.7us of
            # transfer; consts third (first needed by the first matmul) ---
            kts, vs = [], []
            for p in range(PAIRS):
                kt = ktp.tile([128, NBLK, E], f16, tag="kt", name=f"kt{p}")
                v = vp.tile([128, NBLK, E], f16, tag="v", name=f"v{p}")
                nc.sync.dma_start(out=kt[:], in_=ktin[p])
                nc.sync.dma_start(out=v[:], in_=vin[p])
                if p == 0:
                    nc.scalar.dma_start(out=ctri[:], in_=cinA[:])
                    nc.scalar.dma_start(out=cmsk[:], in_=cinB[:])
                kts.append(kt)
                vs.append(v)

            # --- PE warm-up: junk matmuls into the den banks while the
            # input DMAs stream (PE p-state needs ~3us of busy to ramp);
            # the real den prefix matmuls (start=True) reset the banks ---
            for w in range(4):
                nc.tensor.matmul(
                    denbank[w % 2][:], lhsT=triL,
                    rhs=ctri[:].rearrange("p (o c) -> p o c", o=1).broadcast_to(
                        [128, 4, 128]
                    ),
                    start=True, stop=True, skip_group_check=True,
                )

            # --- per-pair g pipeline + wg; den prefix per duo ---
            wgs = {}
            prev_wg = None
            for p in range(PAIRS):
                s1 = s1p.tile([128, NBLK, 32], f16, tag="s1", name=f"s1_{p}")
                s1tt = nc.vector.tensor_tensor(
                    out=s1[:], in0=kts[p][:, :, 0:32], in1=kts[p][:, :, 32:64],
                    op=mybir.AluOpType.add,
                )
                if prev_wg is not None:
                    # order-only: keep the DVE stream chain-major so pair 0's
                    # wg (and thus its whole back-end) is not pushed behind
                    # later pairs' reduce chains by the scheduler
                    add_dep_helper(s1tt.ins, prev_wg.ins, sync=False,
                                   reason="s1 after prev pair wg")
                s2 = s2p.tile([128, NBLK, 16], f16, tag="s2", name=f"s2_{p}")
                nc.vector.tensor_tensor(
                    out=s2[:], in0=s1[:, :, 0:16], in1=s1[:, :, 16:32],
                    op=mybir.AluOpType.add,
                )
                sk = skp.tile([128, NBLK], f32, tag="sk", name=f"sk{p}")
                nc.vector.tensor_reduce(
                    sk[:], s2[:], mybir.AxisListType.X, mybir.AluOpType.add
                )
                nc.scalar.activation(
                    G[:, p, :], sk[:], mybir.ActivationFunctionType.Exp
                )
                wg = wgp.tile([128, NBLK, E], f16, tag="wg", name=f"wg{p}")
                gb = (
                    G[:, p, :]
                    .rearrange("p (k o) -> p k o", o=1)
                    .broadcast_to([128, NBLK, E])
                )
                prev_wg = nc.vector.tensor_tensor(
                    out=wg[:], in0=vs[p][:], in1=gb, op=mybir.AluOpType.mult
                )
                wgs[p] = wg
                # den prefix per pair: start=True on the even pair resets the
                # whole bank (zeroing the odd pair's region), the odd pair
                # accumulates into the zeroed region with start=False
                d, j = p // 2, p % 2
                nc.tensor.matmul(
                    dens[d][:, j, :], lhsT=triL, rhs=G[:, p, :],
                    start=(j == 0), stop=False, skip_group_check=True,
                )

            pss, bss = {}, {}

            pmm_last = {}

            def emit_pmm(p):
                ps = psp.tile([128, NBLK, E], f32, tag="ps", name=f"ps{p}")
                rhs = wgs[p][:]
                nc.tensor.matmul(
                    ps[:, 0:8, :], lhsT=triL, rhs=rhs[:, 0:8, :],
                    start=True, stop=False, skip_group_check=True,
                )
                pmm_last[p] = nc.tensor.matmul(
                    ps[:, 8:16, :], lhsT=triL, rhs=rhs[:, 8:16, :],
                    start=True, stop=False, skip_group_check=True,
                )
                pss[p] = ps

            def emit_extract_scatter(p, split=False):
                bs1 = bs1p.tile([1, NBLK, E], f16, tag="bs1", name=f"bs1_{p}")
                if split:
                    # per-bank extracts: ex-A overlaps the second prefix
                    # matmul on the chain-critical pairs
                    nc.scalar.copy(bs1[:, 0:8, :], pss[p][0:1, 0:8, :])
                    nc.scalar.copy(bs1[:, 8:16, :], pss[p][0:1, 8:16, :])
                else:
                    nc.scalar.copy(bs1[:], pss[p][0:1, :, :])
                bs = bsp.tile([16, E], f16, tag="bs", name=f"bs{p}")
                nc.sync.dma_start(out=bs[:], in_=bs1[:])
                bss[p] = bs

            ots = {}

            ots = {}

            def emit_carry(p, pe_after=None):
                rm = rmp.tile([16, NBLK, E], f16, tag="rm", name=f"rm{p}")
                nc.vector.tensor_tensor(
                    out=rm[:],
                    in0=mask3,
                    in1=bss[p][:].rearrange(
                        "p (o e) -> p o e", o=1
                    ).broadcast_to([16, NBLK, E]),
                    op=mybir.AluOpType.mult,
                )
                cm = nc.tensor.matmul(
                    pss[p][:, 0:8, :], lhsT=ones16, rhs=rm[:, 0:8, :],
                    start=False, stop=True, skip_group_check=True,
                )
                if pe_after is not None:
                    # order-only: keep the tail-critical pair's prefix
                    # matmuls ahead of this pair's carries on PE
                    add_dep_helper(cm.ins, pe_after.ins, sync=False,
                                   reason="carry after tail pair pmm")
                nc.tensor.matmul(
                    pss[p][:, 8:16, :], lhsT=ones16, rhs=rm[:, 8:16, :],
                    start=False, stop=True, skip_group_check=True,
                )
                if p < 2:
                    cw = cwp.tile([128, NBLK, E], f16, tag="cw", name=f"cw{p}")
                    nc.scalar.copy(cw[:], pss[p][:])
                    return cw
                return None

            cws = {}

            def emit_fin(p, half=None):
                if p in ots:
                    ot = ots[p]
                else:
                    ot = otp.tile([128, NBLK, E], f16, tag="ot", name=f"ot{p}")
                    ots[p] = ot
                sl = slice(None) if half is None else slice(8 * half, 8 * half + 8)
                kn = NBLK if half is None else 8
                rb = (
                    r[:, p, sl]
                    .rearrange("p (k o) -> p k o", o=1)
                    .broadcast_to([128, kn, E])
                )
                src_ap = cws[p][:, sl, :] if p < 2 else pss[p][:, sl, :]
                nc.vector.tensor_tensor(
                    out=ot[:, sl, :], in0=src_ap, in1=rb,
                    op=mybir.AluOpType.mult,
                )

            for p in range(3):
                emit_pmm(p)

            # den: row-0 extract, scatter (gpsimd ring), carry, reciprocal
            for d in range(DUOS):
                dbs1 = dbs1p.tile([1, NBLK, 2], f16, tag="dbs1", name=f"dbs1_{d}")
                nc.scalar.copy(
                    dbs1[:].rearrange("p k j -> p j k"), dens[d][0:1]
                )
                dbs = dbsp.tile([16, 2], f16, tag="dbs", name=f"dbs{d}")
                nc.gpsimd.dma_start(out=dbs[:], in_=dbs1[:])
                drm = drmp.tile([16, 2, NBLK], f16, tag="drm", name=f"drm{d}")
                nc.vector.tensor_tensor(
                    out=drm[:],
                    in0=mask3d,
                    in1=dbs[:].rearrange("p (j o) -> p j o", o=1).broadcast_to(
                        [16, 2, NBLK]
                    ),
                    op=mybir.AluOpType.mult,
                )
                nc.tensor.matmul(
                    dens[d][:], lhsT=ones16, rhs=drm[:],
                    start=False, stop=True, skip_group_check=True,
                )
                with nc.allow_low_precision("fp16 reciprocal feeds fp16 output"):
                    nc.vector.reciprocal(
                        r[:, 2 * d : 2 * d + 2, :], dens[d][:]
                    )

            emit_extract_scatter(0, split=True)
            for p in (1, 2):
                emit_extract_scatter(p)
            cws[0] = emit_carry(0)
            cws[1] = emit_carry(1)
            # 4th pair rotates onto pair 0's banks (freed by its drain);
            # emitted here so its matmuls/extract don't queue behind pair 2's
            emit_pmm(3)
            emit_extract_scatter(3, split=True)
            emit_carry(2, pe_after=pmm_last[3])
            emit_carry(3)
            for p in range(3):
                emit_fin(p)
            # tail-critical pair 3: per-half fins/outs pipeline behind the
            # two carry matmuls instead of waiting for both
            emit_fin(3, half=0)
            for p in range(3):
                nc.sync.dma_start(out=outT[p], in_=ots[p][:])
            nc.sync.dma_start(out=outT[3, :, 0:8], in_=ots[3][:, 0:8, :])
            emit_fin(3, half=1)
            nc.sync.dma_start(out=outT[3, :, 8:16], in_=ots[3][:, 8:16, :])

    nc.compile()
    return nc


def _get_compiled():
    global _compiled
    if _compiled is None:
        _compiled = _build()
    return _compiled


def prep_inputs(keys: np.ndarray, values: np.ndarray, w_score: np.ndarray):
    """Host-side reshard: returns in_maps (list of 8 dicts)."""
    keys = np.asarray(keys, dtype=np.float32)
    values = np.asarray(values, dtype=np.float32)
    w = np.asarray(w_score, dtype=np.float32)

    # [B,S,H,E] -> [B*H, NBLK, 128, E], rows reversed within each block
    kt = keys.transpose(0, 2, 1, 3).reshape(B * H, NBLK, 128, E)[:, :, ::-1, :]
    kt = (kt * (-SCALE * w)).astype(np.float16)
    kt = kt.transpose(0, 2, 1, 3)  # [B*H, 128, NBLK, E]

    v = values.transpose(0, 2, 1, 3).reshape(B * H, NBLK, 128, E)[:, :, ::-1, :]
    v = v.astype(np.float16).transpose(0, 2, 1, 3)  # [B*H, 128, NBLK, E]

    tri, cmsk = _consts_host()
    in_maps = []
    for c in range(NCORES):
        sl = slice(PAIRS * c, PAIRS * (c + 1))
        in_maps.append({
            "ktin": np.ascontiguousarray(kt[sl]),
            "vin": np.ascontiguousarray(v[sl]),
            "cinA": tri,
            "cinB": cmsk,
        })
    return in_maps


def assemble_output(results) -> np.ndarray:
    # results[c]["outT"]: [PAIRS, 128, NBLK, E]; s = 128*k + (127-row)
    arr = np.stack([np.asarray(r["outT"]) for r in results])  # [8,P,128,K,E]
    arr = arr.reshape(B * H, 128, NBLK, E)
    arr = arr.transpose(0, 2, 1, 3)[:, :, ::-1, :]  # [BH, k, row_rev, E]
    arr = arr.reshape(B, H, L, E).transpose(0, 2, 1, 3).astype(np.float32)
    return np.ascontiguousarray(arr)


def kernel(queries=None, keys=None, values=None, w_score=None, b_score=None, attn_mask=None, **_):
    global LAST_RESULTS
    from concourse.bass_utils import run_bass_kernel_spmd

    nc = _get_compiled()
    in_maps = prep_inputs(keys, values, w_score)
    res = run_bass_kernel_spmd(nc, in_maps, core_ids=list(range(NCORES)), trace=TRACE)
    LAST_RESULTS = res
    return assemble_output(res.results)


# revision 30
# speedup vs baseline: 1.0552x; 1.0552x over previous
"""MinusAttention kernel for Trainium2 (8 NeuronCores, Bass/Tile).

Math: score[i,j] = (w.q_i - w.k_j + b) / sqrt(E) with causal mask.
Within a softmax row i the w.q_i and b terms are constant across j and
cancel, so

    weights[i,j] = g_j / sum_{j'<=i} g_j',   g_j = exp(-w.k_j / sqrt(E))
    out[i,:]     = (sum_{j<=i} g_j V[j,:]) / (sum_{j<=i} g_j)

i.e. a causal cumulative weighted average of V -- O(S*E) per (b,h) --
and the output does not depend on queries at all.

Device kernel per core (4 of the 32 (b,h) pairs), all fp16 IO:

  s = 128*k + (127 - row): row-REVERSED within each 128-block, so each
  block's running total lands on PSUM row 0.  Prefix = lower-triangular
  fp16 matmul (within block) + per-block carry matmuls.

  Per pair p (its own small DMA so compute starts ~6us in):
    sk = sum_e kt (two fp16 halving adds + fp32 reduce, DVE)
    g = exp(sk) (ACT) -> wg = v*g (DVE) ->
    2x 512-col prefix matmuls into 2 PSUM banks (PE, fp16 1c/row)
    block totals = PSUM row 0: ACT copies [1,(k,e)] -> SBUF fp16,
    SBUF->SBUF DMA scatters them onto 16 partitions (bs)
    rm = strict-lower-mask * bs (DVE 2x, fp16 packed)
    carry matmuls ones16 @ rm accumulate into the same banks
    pairs 0-1: ACT drains PSUM->fp16 cw, out = cw * (1/den) (DVE)
    pairs 2-3: out = PSUM * (1/den) fused on DVE (one hop less; these
    pairs are tail-critical)

  Denominator: per-pair prefix matmuls of g into a per-duo PSUM bank
  (start=True on the even pair resets the bank -- hardware resets the
  WHOLE bank, so duos cannot share one), row-0 extract + scatter, drm
  carry, reciprocal -> r[128, pair, k] fp16.

  Scheduling: all constants are host-baked (one small DMA); 6 junk
  matmuls into the den banks warm the PE p-state during the input
  DMAs; order-only deps keep the DVE stream chain-major; pair 3 reuses
  pair 0's PSUM banks (pool rotation) and its matmul/extract/scatter
  are emitted before the output DMAs to avoid queue head-of-line
  blocking; rm ops for all pairs precede all finals in the DVE stream.

  Measured on trn2: 55.6us (baseline fp32 kernel) -> ~37us; output
  rel err ~6e-4 (fp16 quantization).
"""
import numpy as np

B, L, S, H, E = 4, 2048, 2048, 8, 64
NCORES = 8
PAIRS = (B * H) // NCORES  # 4 (b,h) pairs per core
NBLK = S // 128  # 16 blocks of 128 positions
DUOS = PAIRS // 2  # pairs processed two at a time
SCALE = np.float32(1.0 / np.sqrt(np.float32(E)))

# consts column map: triL is its own [128,128] tensor; the 16-partition
# masks live in a second [16, CWID2] tensor
CW_ONES = 0  # ones16 [16, 0:128]
CW_MASK3 = 128  # mask3 [16, 128:1152]  (k', k, e) strict-lower
CW_MASK3D = 1152  # mask3d [16, 1152:1184]  (k', j, k) strict-lower
CWID2 = 1184

TRACE = False
LAST_RESULTS = None

_compiled = None


def _consts_host():
    tri = np.tril(np.ones((128, 128), np.float16))  # p' >= p (row-reversed)
    c = np.zeros((16, CWID2), dtype=np.float16)
    c[:, CW_ONES : CW_ONES + 128] = 1.0
    kp = np.arange(16)
    m3 = (kp[:, None] < kp[None, :]).astype(np.float16)  # [k', k]
    c[:, CW_MASK3 : CW_MASK3 + NBLK * E] = np.repeat(m3, E, axis=1)
    c[:, CW_MASK3D : CW_MASK3D + 2 * NBLK] = np.concatenate([m3, m3], axis=1)
    return tri, c


def _build():
    from concourse import bacc
    import concourse.mybir as mybir
    import concourse.tile as tile
    from concourse.tile_rust import add_dep_helper

    f16 = mybir.dt.float16
    f32 = mybir.dt.float32
    nc = bacc.Bacc("TRN2", target_bir_lowering=False, debug=False)

    # per-pair tensors, (row, ...) with 2KB contiguous partition lines
    ktin = nc.dram_tensor("ktin", [PAIRS, 128, NBLK, E], f16, kind="ExternalInput")
    vin = nc.dram_tensor("vin", [PAIRS, 128, NBLK, E], f16, kind="ExternalInput")
    cinA = nc.dram_tensor("cinA", [128, 128], f16, kind="ExternalInput")
    cinB = nc.dram_tensor("cinB", [16, CWID2], f16, kind="ExternalInput")
    outT = nc.dram_tensor("outT", [PAIRS, 128, NBLK, E], f16, kind="ExternalOutput")

    with tile.TileContext(nc) as tc:
        with (
            tc.tile_pool(name="const", bufs=1) as cpool,
            tc.tile_pool(name="ktp", bufs=PAIRS) as ktp,
            tc.tile_pool(name="vp", bufs=PAIRS) as vp,
            tc.tile_pool(name="s1p", bufs=2) as s1p,
            tc.tile_pool(name="s2p", bufs=2) as s2p,
            tc.tile_pool(name="skp", bufs=2) as skp,
            tc.tile_pool(name="wgp", bufs=PAIRS) as wgp,
            tc.tile_pool(name="bs1p", bufs=PAIRS) as bs1p,
            tc.tile_pool(name="bsp", bufs=PAIRS) as bsp,
            tc.tile_pool(name="rmp", bufs=PAIRS) as rmp,
            tc.tile_pool(name="dbs1p", bufs=2) as dbs1p,
            tc.tile_pool(name="dbsp", bufs=2) as dbsp,
            tc.tile_pool(name="drmp", bufs=2) as drmp,
            tc.tile_pool(name="cwp", bufs=2) as cwp,
            tc.tile_pool(name="otp", bufs=PAIRS) as otp,
            tc.tile_pool(name="psp", bufs=3, space="PSUM") as psp,
            tc.tile_pool(name="dpsp", bufs=2, space="PSUM") as dpsp,
        ):
            ctri = cpool.tile([128, 128], f16)
            cmsk = cpool.tile([16, CWID2], f16)
            triL = ctri[:]
            ones16 = cmsk[:, CW_ONES : CW_ONES + 128]
            mask3 = cmsk[:, CW_MASK3 : CW_MASK3 + NBLK * E].rearrange(
                "p (k e) -> p k e", k=NBLK
            )
            mask3d = cmsk[:, CW_MASK3D : CW_MASK3D + 2 * NBLK].rearrange(
                "p (j k) -> p j k", j=2
            )

            G = cpool.tile([128, PAIRS, NBLK], f16)
            r = cpool.tile([128, PAIRS, NBLK], f16)
            # full-bank tiles: also the target of PE warm-up matmuls; the
            # real den prefix matmul (start=True) resets the bank
            denbank = [
                dpsp.tile([128, 512], f32, tag="den", name=f"denb{d}")
                for d in range(DUOS)
            ]
            dens = [
                denbank[d][:, 0 : 2 * NBLK].rearrange("p (j k) -> p j k", j=2)
                for d in range(DUOS)
            ]

            # --- inputs: per-pair DMAs so compute starts after ~0.7us of
            # transfer; consts third (first needed by the first matmul) ---
            kts, vs = [], []
            for p in range(PAIRS):
                kt = ktp.tile([128, NBLK, E], f16, tag="kt", name=f"kt{p}")
                v = vp.tile([128, NBLK, E], f16, tag="v", name=f"v{p}")
                nc.sync.dma_start(out=kt[:], in_=ktin[p])
                nc.sync.dma_start(out=v[:], in_=vin[p])
                if p == 0:
                    nc.scalar.dma_start(out=ctri[:], in_=cinA[:])
                    nc.scalar.dma_start(out=cmsk[:], in_=cinB[:])
                kts.append(kt)
                vs.append(v)

            # --- PE warm-up: junk matmuls into the den banks while the
            # input DMAs stream (PE p-state needs ~3us of busy to ramp);
            # the real den prefix matmuls (start=True) reset the banks ---
            for w in range(4):
                nc.tensor.matmul(
                    denbank[w % 2][:], lhsT=triL,
                    rhs=ctri[:].rearrange("p (o c) -> p o c", o=1).broadcast_to(
                        [128, 4, 128]
                    ),
                    start=True, stop=True, skip_group_check=True,
                )

            # --- per-pair g pipeline + wg; den prefix per duo ---
            wgs = {}
            prev_wg = None
            for p in range(PAIRS):
                s1 = s1p.tile([128, NBLK, 32], f16, tag="s1", name=f"s1_{p}")
                s1tt = nc.vector.tensor_tensor(
                    out=s1[:], in0=kts[p][:, :, 0:32], in1=kts[p][:, :, 32:64],
                    op=mybir.AluOpType.add,
                )
                if prev_wg is not None:
                    # order-only: keep the DVE stream chain-major so pair 0's
                    # wg (and thus its whole back-end) is not pushed behind
                    # later pairs' reduce chains by the scheduler
                    add_dep_helper(s1tt.ins, prev_wg.ins, sync=False,
                                   reason="s1 after prev pair wg")
                s2 = s2p.tile([128, NBLK, 16], f16, tag="s2", name=f"s2_{p}")
                nc.vector.tensor_tensor(
                    out=s2[:], in0=s1[:, :, 0:16], in1=s1[:, :, 16:32],
                    op=mybir.AluOpType.add,
                )
                sk = skp.tile([128, NBLK], f32, tag="sk", name=f"sk{p}")
                nc.vector.tensor_reduce(
                    sk[:], s2[:], mybir.AxisListType.X, mybir.AluOpType.add
                )
                nc.scalar.activation(
                    G[:, p, :], sk[:], mybir.ActivationFunctionType.Exp
                )
                wg = wgp.tile([128, NBLK, E], f16, tag="wg", name=f"wg{p}")
                gb = (
                    G[:, p, :]
                    .rearrange("p (k o) -> p k o", o=1)
                    .broadcast_to([128, NBLK, E])
                )
                prev_wg = nc.vector.tensor_tensor(
                    out=wg[:], in0=vs[p][:], in1=gb, op=mybir.AluOpType.mult
                )
                wgs[p] = wg
                # den prefix per pair: start=True on the even pair resets the
                # whole bank (zeroing the odd pair's region), the odd pair
                # accumulates into the zeroed region with start=False
                d, j = p // 2, p % 2
                nc.tensor.matmul(
                    dens[d][:, j, :], lhsT=triL, rhs=G[:, p, :],
                    start=(j == 0), stop=False, skip_group_check=True,
                )

            pss, bss = {}, {}

            pmm_last = {}

            def emit_pmm(p):
                ps = psp.tile([128, NBLK, E], f32, tag="ps", name=f"ps{p}")
                rhs = wgs[p][:]
                nc.tensor.matmul(
                    ps[:, 0:8, :], lhsT=triL, rhs=rhs[:, 0:8, :],
                    start=True, stop=False, skip_group_check=True,
                )
                pmm_last[p] = nc.tensor.matmul(
                    ps[:, 8:16, :], lhsT=triL, rhs=rhs[:, 8:16, :],
                    start=True, stop=False, skip_group_check=True,
                )
                pss[p] = ps

            def emit_extract_scatter(p, split=False):
                bs1 = bs1p.tile([1, NBLK, E], f16, tag="bs1", name=f"bs1_{p}")
                if split:
                    # per-bank extracts: ex-A overlaps the second prefix
                    # matmul on the chain-critical pairs
                    nc.scalar.copy(bs1[:, 0:8, :], pss[p][0:1, 0:8, :])
                    nc.scalar.copy(bs1[:, 8:16, :], pss[p][0:1, 8:16, :])
                else:
                    nc.scalar.copy(bs1[:], pss[p][0:1, :, :])
                bs = bsp.tile([16, E], f16, tag="bs", name=f"bs{p}")
                nc.sync.dma_start(out=bs[:], in_=bs1[:])
                bss[p] = bs

            ots = {}

            ots = {}

            def emit_carry(p, pe_after=None):
                rm = rmp.tile([16, NBLK, E], f16, tag="rm", name=f"rm{p}")
                nc.vector.tensor_tensor(
                    out=rm[:],
                    in0=mask3,
                    in1=bss[p][:].rearrange(
                        "p (o e) -> p o e", o=1
                    ).broadcast_to([16, NBLK, E]),
                    op=mybir.AluOpType.mult,
                )
                cm = nc.tensor.matmul(
                    pss[p][:, 0:8, :], lhsT=ones16, rhs=rm[:, 0:8, :],
                    start=False, stop=True, skip_group_check=True,
                )
                if pe_after is not None:
                    # order-only: keep the tail-critical pair's prefix
                    # matmuls ahead of this pair's carries on PE
                    add_dep_helper(cm.ins, pe_after.ins, sync=False,
                                   reason="carry after tail pair pmm")
                nc.tensor.matmul(
                    pss[p][:, 8:16, :], lhsT=ones16, rhs=rm[:, 8:16, :],
                    start=False, stop=True, skip_group_check=True,
                )
                if p < 2:
                    cw = cwp.tile([128, NBLK, E], f16, tag="cw", name=f"cw{p}")
                    nc.scalar.copy(cw[:], pss[p][:])
                    return cw
                return None

            cws = {}

            def emit_fin(p, half=None):
                if p in ots:
                    ot = ots[p]
                else:
                    ot = otp.tile([128, NBLK, E], f16, tag="ot", name=f"ot{p}")
                    ots[p] = ot
                sl = slice(None) if half is None else slice(8 * half, 8 * half + 8)
                kn = NBLK if half is None else 8
                rb = (
                    r[:, p, sl]
                    .rearrange("p (k o) -> p k o", o=1)
                    .broadcast_to([128, kn, E])
                )
                src_ap = cws[p][:, sl, :] if p < 2 else pss[p][:, sl, :]
                nc.vector.tensor_tensor(
                    out=ot[:, sl, :], in0=src_ap, in1=rb,
                    op=mybir.AluOpType.mult,
                )

            for p in range(3):
                emit_pmm(p)

            # den: row-0 extract, scatter (gpsimd ring), carry, reciprocal
            for d in range(DUOS):
                dbs1 = dbs1p.tile([1, NBLK, 2], f16, tag="dbs1", name=f"dbs1_{d}")
                nc.scalar.copy(
                    dbs1[:].rearrange("p k j -> p j k"), dens[d][0:1]
                )
                dbs = dbsp.tile([16, 2], f16, tag="dbs", name=f"dbs{d}")
                nc.gpsimd.dma_start(out=dbs[:], in_=dbs1[:])
                drm = drmp.tile([16, 2, NBLK], f16, tag="drm", name=f"drm{d}")
                nc.vector.tensor_tensor(
                    out=drm[:],
                    in0=mask3d,
                    in1=dbs[:].rearrange("p (j o) -> p j o", o=1).broadcast_to(
                        [16, 2, NBLK]
                    ),
                    op=mybir.AluOpType.mult,
                )
                nc.tensor.matmul(
                    dens[d][:], lhsT=ones16, rhs=drm[:],
                    start=False, stop=True, skip_group_check=True,
                )
                with nc.allow_low_precision("fp16 reciprocal feeds fp16 output"):
                    nc.vector.reciprocal(
                        r[:, 2 * d : 2 * d + 2, :], dens[d][:]
                    )

            for p in range(3):
                emit_extract_scatter(p)
            cws[0] = emit_carry(0)
            cws[1] = emit_carry(1)
            # 4th pair rotates onto pair 0's banks (freed by its drain);
            # emitted here so its matmuls/extract don't queue behind pair 2's
            emit_pmm(3)
            emit_extract_scatter(3)
            emit_carry(2, pe_after=pmm_last[3])
            emit_carry(3)
            for p in range(3):
                emit_fin(p)
            # tail-critical pair 3: per-half fins/outs pipeline behind the
            # two carry matmuls instead of waiting for both
            emit_fin(3, half=0)
            for p in range(3):
                nc.sync.dma_start(out=outT[p], in_=ots[p][:])
            nc.sync.dma_start(out=outT[3, :, 0:8], in_=ots[3][:, 0:8, :])
            emit_fin(3, half=1)
            nc.sync.dma_start(out=outT[3, :, 8:16], in_=ots[3][:, 8:16, :])

    nc.compile()
    return nc


def _get_compiled():
    global _compiled
    if _compiled is None:
        _compiled = _build()
    return _compiled


def prep_inputs(keys: np.ndarray, values: np.ndarray, w_score: np.ndarray):
    """Host-side reshard: returns in_maps (list of 8 dicts)."""
    keys = np.asarray(keys, dtype=np.float32)
    values = np.asarray(values, dtype=np.float32)
    w = np.asarray(w_score, dtype=np.float32)

    # [B,S,H,E] -> [B*H, NBLK, 128, E], rows reversed within each block
    kt = keys.transpose(0, 2, 1, 3).reshape(B * H, NBLK, 128, E)[:, :, ::-1, :]
    kt = (kt * (-SCALE * w)).astype(np.float16)
    kt = kt.transpose(0, 2, 1, 3)  # [B*H, 128, NBLK, E]

    v = values.transpose(0, 2, 1, 3).reshape(B * H, NBLK, 128, E)[:, :, ::-1, :]
    v = v.astype(np.float16).transpose(0, 2, 1, 3)  # [B*H, 128, NBLK, E]

    tri, cmsk = _consts_host()
    in_maps = []
    for c in range(NCORES):
        sl = slice(PAIRS * c, PAIRS * (c + 1))
        in_maps.append({
            "ktin": np.ascontiguousarray(kt[sl]),
            "vin": np.ascontiguousarray(v[sl]),
            "cinA": tri,
            "cinB": cmsk,
        })
    return in_maps


def assemble_output(results) -> np.ndarray:
    # results[c]["outT"]: [PAIRS, 128, NBLK, E]; s = 128*k + (127-row)
    arr = np.stack([np.asarray(r["outT"]) for r in results])  # [8,P,128,K,E]
    arr = arr.reshape(B * H, 128, NBLK, E)
    arr = arr.transpose(0, 2, 1, 3)[:, :, ::-1, :]  # [BH, k, row_rev, E]
    arr = arr.reshape(B, H, L, E).transpose(0, 2, 1, 3).astype(np.float32)
    return np.ascontiguousarray(arr)


def kernel(queries=None, keys=None, values=None, w_score=None, b_score=None, attn_mask=None, **_):
    global LAST_RESULTS
    from concourse.bass_utils import run_bass_kernel_spmd

    nc = _get_compiled()
    in_maps = prep_inputs(keys, values, w_score)
    res = run_bass_kernel_spmd(nc, in_maps, core_ids=list(range(NCORES)), trace=TRACE)
    LAST_RESULTS = res
    return assemble_output(res.results)
